# revision 1
# baseline (speedup 1.0000x reference)
"""FASTLoss (PSENet/FAST text-detection loss) on 8 Trainium2 cores, v3.

Data-parallel: 16 samples, 2 per core. Host stages all inputs as bf16
(binary gt/mask tensors are exact in bf16; pred rounding is ~0.4% rel,
far inside the 2e-2 tolerance) which halves HBM traffic. On-device work
is decomposed across ALL engines to keep each under the DMA roofline:

  DVE : elementwise products via tensor_tensor bf16 (2x mode) and
        threshold selects via tensor_scalar (4x mode). No DVE accums --
        the TensorScalarPtrReduce path runs 1x on HW (~3.6us/plane).
  ACT : sigmoids + 8 of the masked squares with accum_out ([P,1]
        partials, host sums over partitions).
  Pool: all 10 t*m products (gpsimd tensor_tensor, ~6.3us/plane),
        emitted ahead of the plane loop so the in-order DVE/ACT queues
        never wait on the slow engine.
  PE  : every grand-total reduction as ones-weights matmuls accumulated
        into PSUM half-rows (bases 0/32/64, halves = cols 0:256/256:512);
        host sums the DMA'd PSUM rows.
  OHEM: bisection in p-space on pn = sigmoid(x)*neg (bf16), phase-1 on a
        1/8 subsample only; final full-res pass at hi = mid + DELTA and
        the host fixes the in-gap elements via (k - C_hi) * s^2 with
        s = mid + DELTA/2 (second-order-accurate, ~1e-4 rel).

Math notes (g = gt_text, m = training_mask, both binary; p = sigmoid):
  pos = g*m, neg = m - pos
  dice_text: inter = sum(p*pos)
             union = sum(p^2*pos) + T + n_pos + eps
  T = sum(p^2 over top-k negatives by p), k = min(3*n_pos, n_neg)
  kernels (per plane c): UT = sum(t*m), IK = sum(p*t*m), UP = sum(p^2*m)
             loss_c = 1 - 2*IK/(UP + UT + eps)
"""

import sys

import numpy as np

sys.path.insert(0, "/opt/trn_rl_repo")

import concourse.bass as bass  # noqa: E402
import concourse.tile as tile  # noqa: E402
from concourse import bacc, mybir  # noqa: E402
from concourse.bass_utils import run_bass_kernel_spmd  # noqa: E402

try:
    import ml_dtypes
    BF16_NP = ml_dtypes.bfloat16
except ImportError:  # pragma: no cover
    import jax.numpy as jnp
    BF16_NP = jnp.bfloat16

F32 = mybir.dt.float32
BF16 = mybir.dt.bfloat16
ALU = mybir.AluOpType
ACTF = mybir.ActivationFunctionType

B_PER_CORE = 2
N_CORES = 8
P = 128
FREE = 3200
SUBF = 800        # phase-1 subsample columns (1/4 of FREE, half partitions)
NITER = 7         # phase-1 bisection iterations
DELTA = 0.0105    # threshold safety margin in p-space (covers ~4.5 sigma of
                  # the 1/8-subsample quantile noise + the 1/128 bisect
                  # window; host tie-correction absorbs the gap to ~1e-4)
EPS = 1e-6

# out_psum row map (row = 3*bank + base/32; halves A=0:256 B=256:512):
#  0: npos0|nneg0    1: npos1|nneg1    2: int0|int1
#  3: chi0|chi1      4: p2pos0|p2pos1  5: tsel0|tsel1
#  6..14: UTj|IKj (j=0..8)   15: UT9|IK9   16: UP0|UP1   17: UP2|UP3
#  18: UP4|UP5 (bank2 row 0, reused after its early flush)
PSUM_ROWS = 19
UPX_DVE = (0, 1, 2, 3, 4, 5)   # UP squares on DVE+PE; rest on ACT

# out_stats [128, 16] column map (host sums over partitions)
SC_UP = 0      # +j2 for ACT-UP planes (j2 not in UPX_DVE)
SC_MIDS = 10   # mids copy: rows 0 / 32 hold per-sample phase-1 estimate
SC_NCOL = 16


def build_bass(bench_iters=1, niter=NITER, wb=2, xb=4, tmb=3, pb=2, npool=0, pq_pool=False, ppp_pool=False, upx=UPX_DVE, p2pos_act=False):
    nc = bacc.Bacc("TRN2", target_bir_lowering=False, debug=False)

    pred = nc.dram_tensor("pred", [B_PER_CORE, 6, P, FREE], BF16,
                          kind="ExternalInput").ap()
    gtt = nc.dram_tensor("gt_text", [B_PER_CORE, P, FREE], BF16,
                         kind="ExternalInput").ap()
    gtk = nc.dram_tensor("gt_kernels", [B_PER_CORE, 5, P, FREE], BF16,
                         kind="ExternalInput").ap()
    msk = nc.dram_tensor("training_mask", [B_PER_CORE, P, FREE], BF16,
                         kind="ExternalInput").ap()
    out_psum = nc.dram_tensor("out_psum", [PSUM_ROWS, 512], F32,
                              kind="ExternalOutput").ap()
    out_stats = nc.dram_tensor("out_stats", [P, SC_NCOL], F32,
                               kind="ExternalOutput").ap()

    with tile.TileContext(nc) as tc:
        with (
            tc.tile_pool(name="pin", bufs=1) as pin,
            tc.tile_pool(name="stream", bufs=4) as stream,
            tc.tile_pool(name="work", bufs=wb) as work,
            tc.tile_pool(name="pacc", bufs=1, space="PSUM") as pacc,
            tc.tile_pool(name="pscr", bufs=1, space="PSUM") as pscr,
        ):
            if bench_iters > 1:
                loop_cm = tc.For_i(0, bench_iters, 1)
                loop_cm.__enter__()

            outs = pin.tile([P, SC_NCOL], F32, tag="outs")
            nc.vector.memset(outs, 0.0)

            # constant matmul weights. sample-b bisect state lives on
            # partition 32*b (engine partition bases must be 0/32/64).
            ones1 = pin.tile([P, 1], BF16, tag="ones1")
            nc.vector.memset(ones1, 1.0)
            bm2 = pin.tile([P, 33], BF16, tag="bm2")
            nc.vector.memset(bm2, 0.0)
            nc.vector.memset(bm2[0:64, 0:1], 1.0)
            nc.vector.memset(bm2[64:128, 32:33], 1.0)
            lbc = pin.tile([P, P], F32, tag="lbc")  # striped broadcast
            nc.vector.memset(lbc, 0.0)
            nc.vector.memset(lbc[0:1, 0:64], 1.0)
            nc.vector.memset(lbc[32:33, 64:128], 1.0)
            ab = [pin.tile([P, P], F32, tag=f"ab{b}", name=f"ab{b}")
                  for b in range(B_PER_CORE)]
            for b in range(B_PER_CORE):
                nc.vector.memset(ab[b], 0.0)
                nc.vector.memset(ab[b][32 * b:32 * b + 1, :], 1.0)

            # persistent PSUM accumulation banks
            banks = [pacc.tile([P, 512], F32, tag=f"bank{i}",
                               name=f"bank{i}") for i in range(6)]

            def pe_row(src, row, half):
                """Accumulate sum over partitions of src [P, FREE] into
                out_psum row `row`, half `half` (cols half*256 +: 256),
                via 13 chunked matmuls (last chunk 128 wide). Row 18
                physically reuses bank2 row 0 after its early flush."""
                bank, base = divmod(row if row < 18 else 6, 3)
                dst = banks[bank]
                off = half * 256
                for k in range(13):
                    w = 256 if k < 12 else 128
                    nc.tensor.matmul(
                        dst[base * 32:base * 32 + 1, off:off + w],
                        ones1, src[:, k * 256:k * 256 + w],
                        start=(k == 0), stop=(k == 12))

            # resident tiles
            m_t = [pin.tile([P, FREE], BF16, tag=f"m{b}", name=f"m{b}")
                   for b in range(B_PER_CORE)]
            pn_t = [pin.tile([P, FREE], BF16, tag=f"pn{b}", name=f"pn{b}")
                    for b in range(B_PER_CORE)]
            xg_t = []

            pn2_t = [pin.tile([P, FREE], BF16, tag=f"pn2{b}",
                              name=f"pn2{b}") for b in range(B_PER_CORE)]
            # bisection state (junk on unused partitions is kept finite)
            v2s = pin.tile([P, SUBF], BF16, tag="v2s")
            los = pin.tile([P, 1], F32, tag="los")
            his = pin.tile([P, 1], F32, tag="his")
            mids = pin.tile([P, 1], F32, tag="mids")
            ks = pin.tile([P, 1], F32, tag="ks")
            ksrc = pin.tile([P, 2], F32, tag="ksrc")
            cnt2 = pin.tile([P, 1], F32, tag="cnt2")
            cmp2 = pin.tile([P, 1], mybir.dt.uint32, tag="cmp2")
            his2 = pin.tile([P, 1], F32, tag="his2")
            va = pin.tile([P, 1], F32, tag="va")
            vb = pin.tile([P, 1], F32, tag="vb")
            scrA = pin.tile([P, 256], F32, tag="scrA")
            scr2 = pin.tile([33, 400], F32, tag="scr2")
            nc.vector.memset(los, 0.0)
            nc.vector.memset(his, 1.0)
            nc.vector.memset(mids, 0.5)
            nc.vector.memset(ks, 0.0)
            nc.vector.memset(ksrc, 0.0)
            nc.vector.memset(cnt2, 0.0)
            nc.vector.memset(his2, 0.0)

            planes = [(b, c) for b in range(B_PER_CORE) for c in range(5)]

            # text inputs first (the text phase is the critical-path head)
            for b in range(B_PER_CORE):
                nc.sync.dma_start(out=m_t[b], in_=msk[b])
                x = stream.tile([P, FREE], BF16, tag="x", name="xt", bufs=xb)
                nc.sync.dma_start(out=x, in_=pred[b, 0])
                g = stream.tile([P, FREE], BF16, tag="g", name="gt", bufs=2)
                nc.sync.dma_start(out=g, in_=gtt[b])
                xg_t.append((x, g))

            # Pool t*m pre-pass, emitted with ~3-plane lookahead: bootstrap
            # 3 here, the rest from inside the plane loop
            pool_js = set(range(10 - npool, 10))
            tm_t = {}

            def _pool_tm(j):
                if j not in pool_js:
                    return
                b, c = planes[j]
                t = stream.tile([P, FREE], BF16, tag="t", name="tk", bufs=3)
                nc.sync.dma_start(out=t, in_=gtk[b, c])
                tm = work.tile([P, FREE], BF16, tag="tmp", name="tmp",
                               bufs=2)
                nc.gpsimd.tensor_tensor(out=tm, in0=t, in1=m_t[b],
                                        op=ALU.mult)
                tm_t[j] = tm

            for j in range(10):
                if j < 10 - npool:
                    continue
                if len(tm_t) >= 3:
                    break
                _pool_tm(j)

            # ---------------- text phase ----------------
            for b in range(B_PER_CORE):
                x, g = xg_t[b]
                p = work.tile([P, FREE], BF16, tag="p", name="p", bufs=pb)
                nc.scalar.activation(out=p, in_=x, func=ACTF.Sigmoid)
                posm = work.tile([P, FREE], BF16, tag="posm", name="posm", bufs=pb)
                nc.vector.tensor_tensor(out=posm, in0=g, in1=m_t[b],
                                        op=ALU.mult)
                pe_row(posm, b, 0)             # npos_b
                negm = work.tile([P, FREE], BF16, tag="negm", name="negm")
                nc.vector.tensor_tensor(out=negm, in0=m_t[b], in1=posm,
                                        op=ALU.subtract)
                pe_row(negm, b, 1)             # nneg_b
                nc.vector.tensor_tensor(out=pn_t[b], in0=p, in1=negm,
                                        op=ALU.mult)
                pp = work.tile([P, FREE], BF16, tag="pp", name="pp", bufs=pb)
                nc.vector.tensor_tensor(out=pp, in0=p, in1=posm,
                                        op=ALU.mult)
                pe_row(pp, 2, b)               # intert_b
                ppp = work.tile([P, FREE], BF16, tag="negm", name="ppp",
                                bufs=2)
                nc.vector.tensor_tensor(out=ppp, in0=pp, in1=pp,
                                        op=ALU.mult)
                pe_row(ppp, 4, b)              # p2pos_b

            # ---- bisection chunks (interleaved with kernel planes) ----
            bis_chunks = []

            def _ksetup():
                # npos/nneg totals from bank0 half-rows -> va/vb
                nc.scalar.activation(out=scrA, in_=banks[0][:, 0:256],
                                     func=ACTF.Copy, accum_out=va)
                nc.scalar.activation(out=scrA, in_=banks[0][:, 256:512],
                                     func=ACTF.Copy, accum_out=vb)
                # (npos_b, nneg_b) onto partition 32b
                nc.sync.dma_start(out=ksrc[0:1, 0:1], in_=va[0:1, :])
                nc.sync.dma_start(out=ksrc[0:1, 1:2], in_=vb[0:1, :])
                nc.sync.dma_start(out=ksrc[32:33, 0:1], in_=va[32:33, :])
                nc.sync.dma_start(out=ksrc[32:33, 1:2], in_=vb[32:33, :])
            bis_chunks.append(_ksetup)

            def _ks():
                # ks = min(3*npos, nneg) / 8
                nc.vector.tensor_scalar(
                    out=ks, in0=ksrc[:, 0:1], scalar1=3.0,
                    scalar2=None, op0=ALU.mult)
                nc.vector.tensor_tensor(out=ks, in0=ks,
                                        in1=ksrc[:, 1:2], op=ALU.min)
                nc.vector.tensor_scalar(
                    out=ks, in0=ks, scalar1=0.125,
                    scalar2=None, op0=ALU.mult)
                # subsample: half partitions x first 800 cols
                nc.vector.tensor_copy(v2s[0:64, :], pn_t[0][0:64, 0:SUBF])
                nc.vector.tensor_copy(v2s[64:128, :],
                                      pn_t[1][64:128, 0:SUBF])
            bis_chunks.append(_ks)

            def _pn2(b):
                nc.scalar.activation(out=pn2_t[b], in_=pn_t[b],
                                     func=ACTF.Square)
            bis_chunks.append(lambda: _pn2(0))
            bis_chunks.append(lambda: _pn2(1))

            def _p1_iter():
                midb = pscr.tile([P, 1], F32, tag="midb", name="midb")
                nc.tensor.matmul(midb, lbc, mids, start=True, stop=True)
                sels = work.tile([P, SUBF], BF16, tag="sels", name="sels")
                nc.vector.tensor_scalar(
                    out=sels, in0=v2s, scalar1=midb, scalar2=None,
                    op0=ALU.is_ge)
                cntp = pscr.tile([33, 400], F32, tag="cntp", name="cntp")
                nc.tensor.matmul(cntp, bm2, sels[:, 0:400], start=True,
                                 stop=False)
                nc.tensor.matmul(cntp, bm2, sels[:, 400:800], start=False,
                                 stop=True)
                nc.scalar.activation(out=scr2, in_=cntp, func=ACTF.Copy,
                                     accum_out=cnt2[0:33, :])
                nc.vector.tensor_tensor(out=cmp2, in0=cnt2,
                                        in1=ks, op=ALU.is_ge)
                nc.vector.copy_predicated(out=los, mask=cmp2, data=mids)
                nc.vector.tensor_tensor(out=cmp2, in0=cnt2,
                                        in1=ks, op=ALU.is_lt)
                nc.vector.copy_predicated(out=his, mask=cmp2, data=mids)
                nc.vector.tensor_tensor(out=mids, in0=los,
                                        in1=his, op=ALU.add)
                nc.vector.tensor_scalar_mul(mids, mids, 0.5)
            bis_chunks.extend([_p1_iter] * niter)

            def _hi():
                nc.vector.tensor_scalar(
                    out=his2, in0=mids, scalar1=DELTA,
                    scalar2=None, op0=ALU.add)
                nc.vector.tensor_copy(outs[:, SC_MIDS:SC_MIDS + 1], mids)
            bis_chunks.append(_hi)

            def _final(b):
                hib = pscr.tile([P, 1], F32, tag="midb", name=f"hib{b}")
                nc.tensor.matmul(hib, ab[b], his2, start=True, stop=True)
                sel = work.tile([P, FREE], BF16, tag="sel", name="sel")
                nc.vector.tensor_scalar(
                    out=sel, in0=pn_t[b], scalar1=hib, scalar2=None,
                    op0=ALU.is_ge)
                pe_row(sel, 3, b)              # chi_b
                # tsel = sum(sel * pn^2); pn2 was precomputed early
                w2 = work.tile([P, FREE], BF16, tag="sel", name="w2")
                nc.vector.tensor_tensor(out=w2, in0=sel, in1=pn2_t[b],
                                        op=ALU.mult)
                pe_row(w2, 5, b)               # tsel_b
            bis_chunks.append(lambda: _final(0))
            bis_chunks.append(lambda: _final(1))

            # ---------------- kernel planes (bisection interleaved) -----
            done_banks = set()

            def _flush_bank(i):
                scro = work.tile([P, 512], F32, tag="scro", name="scro")
                nc.scalar.activation(out=scro, in_=banks[i], func=ACTF.Copy)
                for r in range(3):
                    nc.sync.dma_start(
                        out=out_psum[i * 3 + r:i * 3 + r + 1, :],
                        in_=scro[32 * r:32 * r + 1, :])
                done_banks.add(i)

            emitted = 0
            xk_pre = {}

            def _xk_dma(j):
                bj, cj = planes[j]
                xk = stream.tile([P, FREE], BF16, tag="x", name="xk",
                                 bufs=xb)
                nc.sync.dma_start(out=xk, in_=pred[bj, cj + 1])
                xk_pre[j] = xk

            _xk_dma(0)
            for j, (b, c) in enumerate(planes):
                if j + 3 < len(planes) and (j + 3) not in tm_t:
                    _pool_tm(j + 3)
                if j + 1 < len(planes):
                    _xk_dma(j + 1)
                xk = xk_pre.pop(j)
                j2 = b * 5 + c

                pk = work.tile([P, FREE], BF16, tag="p", name="pk", bufs=pb)
                nc.scalar.activation(out=pk, in_=xk, func=ACTF.Sigmoid)
                if j2 not in tm_t:
                    t = stream.tile([P, FREE], BF16, tag="t", name="tk",
                                    bufs=3)
                    nc.sync.dma_start(out=t, in_=gtk[b, c])
                    tmj = work.tile([P, FREE], BF16, tag="tm", name="tm",
                                    bufs=tmb)
                    nc.vector.tensor_tensor(out=tmj, in0=t, in1=m_t[b],
                                            op=ALU.mult)
                    tm_t[j2] = tmj
                ikv = work.tile([P, FREE], BF16, tag="posm", name="ikv", bufs=pb)
                nc.vector.tensor_tensor(out=ikv, in0=tm_t[j2], in1=pk,
                                        op=ALU.mult)
                ut_row = 6 + j2
                pe_row(tm_t[j2], ut_row, 0)    # UT_j2
                pe_row(ikv, ut_row, 1)         # IK_j2
                pmtag = "pmkp" if (j2 in upx and pq_pool) else "pp"
                pmk = work.tile([P, FREE], BF16, tag=pmtag, name="pmk",
                                bufs=2 if pmtag == "pmkp" else pb)
                nc.vector.tensor_tensor(out=pmk, in0=pk, in1=m_t[b],
                                        op=ALU.mult)
                if j2 in upx:
                    pq = work.tile([P, FREE], BF16, tag="pq", name="pq",
                                   bufs=2)
                    if pq_pool:
                        nc.gpsimd.tensor_tensor(out=pq, in0=pmk, in1=pmk,
                                                op=ALU.mult)
                    else:
                        nc.vector.tensor_tensor(out=pq, in0=pmk, in1=pmk,
                                                op=ALU.mult)
                    ui = upx.index(j2)
                    pe_row(pq, (16, 17, 18)[ui // 2], ui % 2)
                else:
                    nc.scalar.activation(
                        out=pmk, in_=pmk, func=ACTF.Square,
                        accum_out=outs[:, SC_UP + j2:SC_UP + j2 + 1])
                # flush banks as their last accumulation completes.
                # bank2 (rows 6-8) flushes early; its row 0 is then reused
                # for UP4|UP5 (out_psum row 18, flushed at the end).
                if j2 == 0:
                    _flush_bank(0)   # npos/nneg/int rows (text)
                elif j2 == 3:
                    _flush_bank(2)   # rows 6-8 (j2 0-2)
                elif j2 == 6:
                    _flush_bank(3)   # rows 9-11 (j2 3-5)

                target = min(len(bis_chunks), (j + 1) * 2)
                while emitted < target:
                    bis_chunks[emitted]()
                    emitted += 1
            while emitted < len(bis_chunks):
                bis_chunks[emitted]()
                emitted += 1

            # ---------------- output ----------------
            for i in (4, 5, 1, 3, 2, 0):
                if i not in done_banks:
                    _flush_bank(i)
            # second flush of bank2 row 0 (UP4|UP5)
            scr18 = work.tile([P, 512], F32, tag="scro", name="scr18")
            nc.scalar.activation(out=scr18[0:64, :], in_=banks[2][0:64, :],
                                 func=ACTF.Copy)
            nc.sync.dma_start(out=out_psum[18:19, :], in_=scr18[0:1, :])
            nc.sync.dma_start(out=out_stats, in_=outs)

            if bench_iters > 1:
                loop_cm.__exit__(None, None, None)

    nc.compile()
    return nc


_NC_CACHE = None


def _get_nc():
    global _NC_CACHE
    if _NC_CACHE is None:
        _NC_CACHE = build_bass()
    return _NC_CACHE


def _to_bf16(x):
    # vectorized round-to-nearest-even f32 -> bf16 (ml_dtypes astype is slow)
    u = np.ascontiguousarray(np.asarray(x, dtype=np.float32)).view(np.uint32)
    r = u + np.uint32(0x7FFF) + ((u >> np.uint32(16)) & np.uint32(1))
    return (r >> np.uint32(16)).astype(np.uint16).view(BF16_NP)


def make_in_maps(pred, gt_text, gt_kernels, training_mask):
    pred = _to_bf16(pred)
    gt_text = _to_bf16(gt_text)
    gt_kernels = _to_bf16(gt_kernels)
    training_mask = _to_bf16(training_mask)
    in_maps = []
    for core in range(N_CORES):
        s = slice(core * B_PER_CORE, (core + 1) * B_PER_CORE)
        in_maps.append({
            "pred": np.ascontiguousarray(pred[s]).reshape(
                B_PER_CORE, 6, P, FREE),
            "gt_text": np.ascontiguousarray(gt_text[s]).reshape(
                B_PER_CORE, P, FREE),
            "gt_kernels": np.ascontiguousarray(gt_kernels[s]).reshape(
                B_PER_CORE, 5, P, FREE),
            "training_mask": np.ascontiguousarray(training_mask[s]).reshape(
                B_PER_CORE, P, FREE),
        })
    return in_maps


def combine(core_outs):
    """core_outs: list of 8 (out_psum [18,512], out_stats [128,16])
    -> (loss, loss_text, loss_kernels)."""
    text_losses = []
    kernel_losses = []
    A, B = slice(0, 256), slice(256, 512)
    for op, os_ in core_outs:
        op = np.asarray(op, dtype=np.float64)
        os_ = np.asarray(os_, dtype=np.float64)
        npos = [op[0, A].sum(), op[1, A].sum()]
        nneg = [op[0, B].sum(), op[1, B].sum()]
        intert = [op[2, A].sum(), op[2, B].sum()]
        chi = [op[3, A].sum(), op[3, B].sum()]
        p2pos = [op[4, A].sum(), op[4, B].sum()]
        tsel = [op[5, A].sum(), op[5, B].sum()]
        ut = {}
        ik = {}
        for j2 in range(10):
            r = 6 + j2
            ut[j2] = op[r, A].sum()
            ik[j2] = op[r, B].sum()
        up = {}
        for j2 in UPX_DVE:
            ui = UPX_DVE.index(j2)
            up[j2] = op[(16, 17, 18)[ui // 2], A if ui % 2 == 0 else B].sum()
        for j2 in range(10):
            if j2 not in up:
                up[j2] = os_[:, SC_UP + j2].sum()
        mids = [os_[0, SC_MIDS], os_[32, SC_MIDS]]

        for b in range(B_PER_CORE):
            k = min(3.0 * npos[b], nneg[b])
            s = mids[b] + DELTA / 2.0
            T = tsel[b] + (k - chi[b]) * s * s
            union = p2pos[b] + T + npos[b] + EPS
            text_losses.append(1.0 - 2.0 * intert[b] / union)
            for c in range(5):
                j2 = b * 5 + c
                union_k = up[j2] + ut[j2] + EPS
                kernel_losses.append(1.0 - 2.0 * ik[j2] / union_k)
    loss_text = float(np.mean(text_losses))
    loss_kernels = float(np.mean(kernel_losses))
    loss = loss_kernels + 0.5 * loss_text
    return (np.float32(loss), np.float32(loss_text), np.float32(loss_kernels))


def kernel(pred, gt_text, gt_kernels, training_mask):
    nc = _get_nc()
    in_maps = make_in_maps(pred, gt_text, gt_kernels, training_mask)
    res = run_bass_kernel_spmd(nc, in_maps, core_ids=list(range(N_CORES)))
    core_outs = [(res.results[i]["out_psum"], res.results[i]["out_stats"])
                 for i in range(N_CORES)]
    return combine(core_outs)


if __name__ == "__main__":
    rng = np.random.default_rng(0)
    B, C, H, W = 16, 6, 640, 640
    pred = rng.standard_normal((B, C, H, W), dtype=np.float32)
    gt_text = (rng.random((B, 1, H, W)) > 0.9).astype(np.float32)
    gt_kernels = (rng.random((B, C - 1, H, W)) > 0.9).astype(np.float32)
    training_mask = (rng.random((B, 1, H, W)) > 0.05).astype(np.float32)
    print(kernel(pred, gt_text, gt_kernels, training_mask))

